# revision 1
# baseline (speedup 1.0000x reference)
"""AxialAttention Trainium2 kernel (8-core data-parallel over batch).

Per image: qkv = x @ qkv_w + alpha*img; per head (16, dh=64) axial-roped
q,k; scores along W per row (no softmax); v row-summed; GroupNorm per
(b, head); output projection.

Algebraic simplifications (exact):
  - per-head gamma scale on k is removed by GroupNorm -> dropped.
  - height-half rope rotations cancel in q.k (same row) -> rope only on
    width-half features (32 of 64 per head).
  - v only needed row-summed: vsum = (A @ x) @ Wv + alpha*(A @ img).
  - GroupNorm folded into the output projection:
      y = (s*of)^T @ wo + ones ⊗ c,  c = sum_h t_h * rowsum_h(wo)
    with s,t the per-(image,head) scale/shift.

Performance structure:
  - all large matmuls in bf16 (1 cycle/row) or f32r with N>=512;
    fp32 matmuls (4 cycles/row) eliminated.
  - phase A: x row-sums (f32r). phase B (per 112-token tile, software
    pipelined): img row-sums, x transpose, qkv+alpha*img via PE, rope,
    q/k feature-major strips; per-image scores into packed PSUM.
  - vsum finalize, then phase C per image: attention output, stats,
    normalization fold, output projection, PSUM->DRAM store.
"""

import math
import sys

import numpy as np

for _p in ("/opt/trn_rl_repo", "/root/.axon_site/_ro/trn_rl_repo"):
    if _p not in sys.path:
        sys.path.append(_p)

import concourse.bacc as bacc
import concourse.mybir as mybir
from concourse import bass_isa, tile
from concourse.bass_utils import run_bass_kernel_spmd

F32 = mybir.dt.float32
F32R = mybir.dt.float32r
BF16 = mybir.dt.bfloat16
ALU = mybir.AluOpType
ACTF = mybir.ActivationFunctionType

HEADS = 16
DH = 64
H = W = 28
HID = 1024
B_FULL = 32
N_CORES = 8
B_CORE = B_FULL // N_CORES          # 4 images per core
TOK = B_CORE * H * W                # 3136 tokens per core
TT = 112                            # tokens per tile (4 rows)
NTILES = TOK // TT                  # 28
TPI = H * W                         # 784 tokens per image
JPI = TPI // TT                     # 7 tiles per image
ALPHA = 1.0 - math.tanh(math.pi * 6.0 / 12.0)
EPS = 1e-5
NGRP = float(H * W * DH)

KBASE = 8 * TPI                     # start of padded k strips in qk_all
KCOLS = 28 * 32                     # 896: k strip row-blocks padded to 32

DEBUG = False

_CACHE = {}


def _build_program(gn_w, gn_b):
    nc = bacc.Bacc("TRN2", target_bir_lowering=False, debug=False,
                   num_devices=N_CORES)

    x_d = nc.dram_tensor("x", [TOK, HID], BF16, kind="ExternalInput").ap()
    img_d = nc.dram_tensor("img", [TOK, HID], BF16, kind="ExternalInput").ap()
    wqk_d = nc.dram_tensor("wqk", [HID, 2 * HID], BF16, kind="ExternalInput").ap()
    wv_d = nc.dram_tensor("wv", [HID, HID], BF16, kind="ExternalInput").ap()
    wo_d = nc.dram_tensor("wo", [HID, HID], BF16, kind="ExternalInput").ap()
    at_d = nc.dram_tensor("at", [TPI, 32], BF16, kind="ExternalInput").ap()
    idnb_d = nc.dram_tensor("idnb", [128, 128], BF16, kind="ExternalInput").ap()
    cst_d = nc.dram_tensor("cst", [TPI, HID], BF16, kind="ExternalInput").ap()
    hm_d = nc.dram_tensor("hm", [HID, 64], BF16, kind="ExternalInput").ap()
    y_d = nc.dram_tensor("y", [TOK, HID], F32, kind="ExternalOutput").ap()
    if DEBUG:
        dbgb_d = nc.dram_tensor("dbgb", [1536, HID], BF16,
                                kind="ExternalOutput").ap()
        dbgf_d = nc.dram_tensor("dbgf", [512, 1024], F32,
                                kind="ExternalOutput").ap()

    from contextlib import ExitStack
    with ExitStack() as ctx:
        tc = ctx.enter_context(tile.TileContext(nc))
        constp = ctx.enter_context(tc.tile_pool(name="const", bufs=1))
        cstp = ctx.enter_context(tc.tile_pool(name="cst", bufs=1))
        wqkp = ctx.enter_context(tc.tile_pool(name="wqk", bufs=1))
        wop = ctx.enter_context(tc.tile_pool(name="wo", bufs=1))
        whsp = ctx.enter_context(tc.tile_pool(name="whs", bufs=1))
        xsp = ctx.enter_context(tc.tile_pool(name="xs", bufs=1))
        qkap = ctx.enter_context(tc.tile_pool(name="qka", bufs=1))
        stsp = ctx.enter_context(tc.tile_pool(name="sts", bufs=1))
        vsump = ctx.enter_context(tc.tile_pool(name="vsum", bufs=1))

        # ---------------- constants ----------------
        idnb = constp.tile([128, 128], BF16, tag="idnb")
        nc.sync.dma_start(idnb[:], idnb_d[:])
        at32 = constp.tile([TT, 7 * 32], BF16, tag="at32")
        nc.sync.dma_start(at32[:].rearrange("p (a c) -> p a c", c=32),
                          at_d[:].rearrange("(a p) c -> p a c", a=JPI))
        hm_sb = constp.tile([128, 8 * 64], BF16, tag="hm")
        nc.sync.dma_start(hm_sb[:].rearrange("p (a c) -> p a c", c=64),
                          hm_d[:].rearrange("(a p) c -> p a c", a=8))
        gw = constp.tile([128, 8], F32, tag="gw")
        gb = constp.tile([128, 8], F32, tag="gb")
        for p in range(8):
            nc.gpsimd.memset(gw[0:64, p:p + 1], float(gn_w[2 * p]))
            nc.gpsimd.memset(gw[64:128, p:p + 1], float(gn_w[2 * p + 1]))
            nc.gpsimd.memset(gb[0:64, p:p + 1], float(gn_b[2 * p]))
            nc.gpsimd.memset(gb[64:128, p:p + 1], float(gn_b[2 * p + 1]))
        epsb = constp.tile([128, 1], F32, tag="epsb")
        nc.gpsimd.memset(epsb[:], EPS)
        ones112 = constp.tile([1, TT], BF16, tag="ones112")
        nc.gpsimd.memset(ones112[:], 1.0)
        ones11 = constp.tile([1, 1], F32, tag="ones11")
        nc.gpsimd.memset(ones11[:], 1.0)
        scmask = constp.tile([128, 392], BF16, tag="scmask")
        nc.gpsimd.memset(scmask[:], 0.0)
        for blk in range(4):
            qq = blk % 2
            v = scmask[32 * blk:32 * (blk + 1), :].rearrange(
                "p (g c) -> p g c", c=56)[:, :, 28 * qq:28 * (qq + 1)]
            nc.gpsimd.memset(v[:], 1.0)

        # rope tables: one tile, 7 per-image-tile column blocks
        cstt = cstp.tile([TT, JPI * HID], BF16, tag="cst")
        nc.sync.dma_start(cstt[:].rearrange("p (a c) -> p a c", c=HID),
                          cst_d[:].rearrange("(a p) c -> p a c", a=JPI))
        cst_sb = [cstt[:, HID * j:HID * (j + 1)] for j in range(JPI)]

        # ---------------- phase A: x row-sums (bf16) ----------------
        abctx = ctx.enter_context(ExitStack())
        xbp = abctx.enter_context(tc.tile_pool(name="xb", bufs=2))
        imgp = abctx.enter_context(tc.tile_pool(name="imgin", bufs=2))
        xtsp = abctx.enter_context(tc.tile_pool(name="xts", bufs=2))
        qkcp = abctx.enter_context(tc.tile_pool(name="qkc", bufs=2))
        ropep = abctx.enter_context(tc.tile_pool(name="rope", bufs=2))
        xs_sb = xsp.tile([128, HID], BF16, tag="xs_sb")
        with tc.tile_pool(name="pxrs", bufs=1, space="PSUM") as pxrs:
            xrs = pxrs.tile([128, HID], F32, tag="xrs")
            for g4 in range(NTILES // 4):
                xa = xbp.tile([TT, 4 * HID], BF16, tag="xb4",
                              name=f"xab{g4}")
                src = x_d[TT * 4 * g4:TT * 4 * (g4 + 1), :].rearrange(
                    "(a p) c -> p a c", a=4)
                nc.sync.dma_start(
                    xa[:].rearrange("p (a c) -> p a c", c=HID), src)
                for h in range(4):
                    i = 4 * g4 + h
                    j, b = i % JPI, i // JPI
                    for n in range(2):
                        cs = slice(HID * h + 512 * n,
                                   HID * h + 512 * (n + 1))
                        nc.tensor.matmul(
                            xrs[32 * b:32 * b + 32, 512 * n:512 * (n + 1)],
                            at32[:, 32 * j:32 * (j + 1)],
                            xa[:, cs], tile_position=(0, 32 * b),
                            start=(j == 0), stop=(j == JPI - 1),
                            skip_group_check=True)
            nc.vector.tensor_copy(xs_sb[:], xrs[:])
        if DEBUG:
            nc.sync.dma_start(dbgb_d[128:256, :], xs_sb[:])

        # weights (bf16 from host) + wo head-sums
        wqk_sb = []
        wo_sb = []
        for k in range(8):
            t = wqkp.tile([128, 2 * HID], BF16, tag=f"wqk{k}",
                          name=f"wqk_sb{k}")
            nc.sync.dma_start(t[:], wqk_d[128 * k:128 * (k + 1), :])
            wqk_sb.append(t)
        for k in range(8):
            t = wop.tile([128, HID], BF16, tag=f"wo{k}", name=f"wo_sb{k}")
            nc.sync.dma_start(t[:], wo_d[128 * k:128 * (k + 1), :])
            wo_sb.append(t)
        wo_hs = whsp.tile([64, HID], BF16, tag="wohs")
        with tc.tile_pool(name="pwhs", bufs=1, space="PSUM") as pwhs:
            whs_ps = pwhs.tile([64, HID], F32, tag="whs")
            for nn in range(2):
                cs = slice(512 * nn, 512 * (nn + 1))
                for k in range(8):
                    nc.tensor.matmul(whs_ps[:, cs],
                                     hm_sb[:, 64 * k:64 * (k + 1)],
                                     wo_sb[k][:, cs],
                                     start=(k == 0), stop=(k == 7))
            nc.vector.tensor_copy(wo_hs[:], whs_ps[:])

        # vs = rowsum(x) @ wv, precomputed before phase B (psum scope closes)
        vs_sb = xsp.tile([128, HID], F32, tag="vs_sb")
        with tc.tile_pool(name="finw", bufs=2) as finwp, \
             tc.tile_pool(name="ptrf", bufs=2, space="PSUM") as ptrf, \
             tc.tile_pool(name="pvs", bufs=1, space="PSUM") as pvs:
            xsT = finwp.tile([128, HID], BF16, tag="xsT", name="xsT")
            for k in range(8):
                trf = ptrf.tile([128, 128], BF16, tag="trf")
                nc.tensor.transpose(trf[:],
                                    xs_sb[:, 128 * k:128 * (k + 1)],
                                    idnb[:])
                nc.vector.tensor_copy(xsT[:, 128 * k:128 * (k + 1)], trf[:])
            vs = pvs.tile([128, HID], F32, tag="vs")
            for k in range(8):
                wvt = finwp.tile([128, HID], BF16, tag="wvt")
                nc.sync.dma_start(wvt[:], wv_d[128 * k:128 * (k + 1), :])
                for n in range(2):
                    cs = slice(512 * n, 512 * (n + 1))
                    nc.tensor.matmul(vs[:, cs],
                                     xsT[:, 128 * k:128 * (k + 1)],
                                     wvt[:, cs],
                                     start=(k == 0), stop=(k == 7))
            nc.vector.tensor_copy(vs_sb[:], vs[:])

        # ---------------- phase B: main per-tile pipeline ----------------
        is_sb = xsp.tile([128, HID], BF16, tag="is_sb")
        qk_all = {}
        st_sb = {}
        for b in range(B_CORE):
            st_sb[b] = [stsp.tile([128, TPI], BF16, tag=f"st{b}_{p}",
                                  name=f"st{b}_{p}") for p in range(8)]

        def emit_front(i, st):
            """DMA + cast + x-transpose + img row-sum for tile i."""
            j, b = i % JPI, i // JPI
            rs = slice(TT * i, TT * (i + 1))
            st["img"] = it = imgp.tile([TT, HID], BF16, tag="ib",
                                       name=f"it{i}")
            nc.sync.dma_start(it[:], img_d[rs, :])
            for n in range(2):
                cs = slice(512 * n, 512 * (n + 1))
                nc.tensor.matmul(irs[32 * b:32 * b + 32, cs],
                                 at32[:, 32 * j:32 * (j + 1)],
                                 it[:, cs], tile_position=(0, 32 * b),
                                 start=(j == 0), stop=(j == JPI - 1),
                                 skip_group_check=True)
            xb = xbp.tile([TT, HID], BF16, tag="xb")
            nc.sync.dma_start(xb[:], x_d[rs, :])
            st["xts"] = xts = xtsp.tile([128, 8 * TT], BF16, tag="xts", name=f"xts{i}")
            for g in range(2):
                tr = ptr.tile([128, 4 * TT], BF16, tag="tr")
                for c in range(4):
                    k = 4 * g + c
                    nc.tensor.transpose(tr[:, TT * c:TT * (c + 1)],
                                        xb[:, 128 * k:128 * (k + 1)],
                                        idnb[0:TT, 0:TT])
                nc.scalar.copy(
                    xts[:, 4 * TT * g:4 * TT * (g + 1)], tr[:])

        def emit_back(i, st):
            """qkv + rope + q/k strips for tile i; scores at image end."""
            j, b = i % JPI, i // JPI
            if j == 0:
                qk_all[b] = qkap.tile([128, KBASE + 8 * KCOLS], BF16,
                                      tag="qka", name=f"qka{b}")
                # zero the 4-col pads of each padded k strip
                for p in range(8):
                    pads = qk_all[b][:, KBASE + KCOLS * p:
                                     KBASE + KCOLS * (p + 1)].rearrange(
                        "p (r c) -> p r c", c=32)[:, :, 28:32]
                    nc.vector.memset(pads[:], 0.0)
            xts, it = st["xts"], st["img"]
            qkc = qkcp.tile([TT, 2 * HID], BF16, tag="qkc")
            for n in range(4):
                pq = ppq.tile([TT, 512], F32, tag="pq")
                for k in range(8):
                    nc.tensor.matmul(pq[:], xts[:, TT * k:TT * (k + 1)],
                                     wqk_sb[k][:, 512 * n:512 * (n + 1)],
                                     start=(k == 0), stop=(k == 7))
                ics = slice(512 * (n % 2), 512 * (n % 2 + 1))
                cs = slice(512 * n, 512 * (n + 1))
                nc.vector.scalar_tensor_tensor(qkc[:, cs], it[:, ics],
                                               ALPHA, pq[:],
                                               ALU.mult, ALU.add)
            # rope on width-half features, q half then k half
            ct = cst_sb[j]
            cv = ct[:, 0:512].rearrange("p (h d) -> p h d", d=32)
            sv = ct[:, 512:1024].rearrange("p (h d) -> p h d", d=32)
            for hh in range(2):
                qv = qkc[:, HID * hh:HID * (hh + 1)].rearrange(
                    "p (h d) -> p h d", d=64)[:, :, 32:64]
                t1 = ropep.tile([TT, 512], BF16, tag=f"t1_{hh}")
                t1v = t1[:].rearrange("p (h d) -> p h d", d=32)
                t2 = ropep.tile([TT, 512], BF16, tag=f"t2_{hh}")
                t2v = t2[:].rearrange("p (h d) -> p h d", d=32)
                nc.vector.tensor_tensor(t1v[:], qv[:], cv[:], op=ALU.mult)
                nc.vector.tensor_tensor(t2v[:, :, 0:16], qv[:, :, 16:32],
                                        sv[:, :, 0:16], op=ALU.mult)
                nc.vector.tensor_tensor(t2v[:, :, 16:32], qv[:, :, 0:16],
                                        sv[:, :, 16:32], op=ALU.mult)
                nc.vector.tensor_tensor(qv[:], t1v[:], t2v[:], op=ALU.add)
            # transpose to feature-major strips in qk_all
            qka = qk_all[b]
            for n in range(4):
                tr = ptr.tile([128, 4 * TT], BF16, tag="tr")
                for c in range(4):
                    cc = 4 * n + c
                    nc.tensor.transpose(tr[:, TT * c:TT * (c + 1)],
                                        qkc[:, 128 * cc:128 * (cc + 1)],
                                        idnb[0:TT, 0:TT])
                if n < 2:
                    dst = qka[:, TPI * 4 * n:TPI * 4 * (n + 1)].rearrange(
                        "p (c t) -> p c t", t=TPI)[:, :, TT * j:TT * (j + 1)]
                    if n % 2 == 0:
                        nc.vector.tensor_copy(dst[:], tr[:])
                    else:
                        nc.scalar.copy(dst[:], tr[:])
                else:
                    srcv = tr[:].rearrange("p (s r c) -> p s r c", s=4, c=28)
                    for c in range(4):
                        p = 4 * (n - 2) + c
                        dst = qka[:, KBASE + KCOLS * p + 128 * j:
                                  KBASE + KCOLS * p + 128 * (j + 1)].rearrange(
                            "p (r c) -> p r c", c=32)[:, :, 0:28]
                        if c % 2 == 0:
                            nc.vector.tensor_copy(dst[:], srcv[:, c])
                        else:
                            nc.scalar.copy(dst[:], srcv[:, c])
            if DEBUG and i == 0:
                nc.sync.dma_start(dbgb_d[384:496, :], qkc[:, 0:HID])
                nc.sync.dma_start(dbgb_d[1409:1521, 0:896], xts[0:112, :])
            if j == JPI - 1:
                if DEBUG and b == 0:
                    nc.sync.dma_start(dbgb_d[512:640, :],
                                      qk_all[0][:, 0:HID])
                    nc.sync.dma_start(dbgb_d[640:768, :],
                                      qk_all[0][:, KBASE:KBASE + HID])
                emit_scores(b)
                if DEBUG and b == 0:
                    nc.sync.dma_start(dbgb_d[768:896, 0:TPI],
                                      st_sb[0][0][:, :])

        def emit_scores(b):
            qka = qk_all[b]
            for p in range(8):
                kofs = KBASE + KCOLS * p
                qofs = TPI * p
                for half in range(2):
                    sc = psc.tile([128, 392], F32, tag="sc")
                    for hn in range(2):
                        hb = 64 * hn
                        for g2 in range(7):
                            g = 7 * half + g2
                            nc.tensor.matmul(
                                sc[hb:hb + 64, 56 * g2:56 * (g2 + 1)],
                                qka[hb:hb + 64, kofs + 64 * g:kofs + 64 * (g + 1)],
                                qka[hb:hb + 64, qofs + 56 * g:qofs + 56 * (g + 1)],
                                tile_position=(hb, hb),
                                start=True, stop=True, skip_group_check=True)
                    nc.vector.tensor_tensor(
                        st_sb[b][p][:, 392 * half:392 * (half + 1)],
                        sc[:], scmask[:], op=ALU.mult)

        with tc.tile_pool(name="pirs", bufs=1, space="PSUM") as pirs, \
             tc.tile_pool(name="ppq", bufs=2, space="PSUM") as ppq, \
             tc.tile_pool(name="ptr", bufs=2, space="PSUM") as ptr, \
             tc.tile_pool(name="psc", bufs=2, space="PSUM") as psc:
            irs = pirs.tile([128, HID], F32, tag="irs")
            states = {}
            for i in range(NTILES + 1):
                if i < NTILES:
                    states[i] = {}
                    emit_front(i, states[i])
                if i >= 1:
                    emit_back(i - 1, states.pop(i - 1))
            nc.vector.tensor_copy(is_sb[:], irs[:])
            if DEBUG:
                nc.sync.dma_start(dbgb_d[256:384, :], is_sb[:])
        abctx.close()

        # ---------------- vsum = vs + alpha * rowsum(img) ----------------
        vsum = vsump.tile([128, HID], BF16, tag="vsum")
        for n in range(2):
            cs = slice(512 * n, 512 * (n + 1))
            nc.vector.scalar_tensor_tensor(vsum[:, cs], is_sb[:, cs],
                                           ALPHA, vs_sb[:, cs],
                                           ALU.mult, ALU.add)
        if DEBUG:
            nc.sync.dma_start(dbgb_d[0:128, :], vsum[:])

        # ---------------- phase C: attention out + GN fold + y ----------
        def emit_c_front(b, st):
            tb = tbp.tile([128, HID], BF16, tag="tb")
            nc.vector.memset(tb[:], 0.0)
            for jj in range(4):
                nc.vector.tensor_copy(tb[32 * jj:32 * jj + 28, :],
                                      vsum[32 * b:32 * b + 28, :])
            statb = statp.tile([128, 16], F32, tag="statb")
            nc.gpsimd.memset(statb[:], 0.0)
            st["of"] = []
            for p in range(8):
                # halves at 512-aligned offsets so matmuls stay in-bank
                ot = pot.tile([128, 1024], F32, tag="ot")
                for hn in range(2):
                    n = 2 * p + hn
                    hb = 64 * hn
                    for half in range(2):
                        nc.tensor.matmul(
                            ot[hb:hb + 64, 512 * half:512 * half + 392],
                            tb[hb:hb + 64, 64 * n:64 * (n + 1)],
                            st_sb[b][p][hb:hb + 64,
                                        392 * half:392 * (half + 1)],
                            tile_position=(hb, hb),
                            start=True, stop=True, skip_group_check=True)
                otv = ot[:].rearrange("p (h c) -> p h c", c=512)[:, :, 0:392]
                of = ofp.tile([128, TPI], BF16, tag=f"of{p}", name=f"of{b}_{p}")
                st["of"].append(of)
                ofv = of[:].rearrange("p (h c) -> p h c", c=392)
                nc.scalar.activation(ofv[:], otv[:], ACTF.Copy,
                                     accum_out=statb[:, p:p + 1])
                sq = sqp.tile([128, TPI], BF16, tag="sq")
                sqv = sq[:].rearrange("p (h c) -> p h c", c=392)
                nc.scalar.activation(sqv[:], otv[:], ACTF.Square,
                                     accum_out=statb[:, 8 + p:9 + p])
            # per-head stats: channels=128 all-reduce with odd-head
            # partials shuffled into extra columns (rows 64:128 zeroed)
            statb2 = statp.tile([128, 32], F32, tag="statb2")
            nc.gpsimd.memset(statb2[64:128, :], 0.0)
            nc.scalar.copy(statb2[0:64, 0:16], statb[0:64, :])
            nc.scalar.copy(statb2[0:64, 16:32], statb[64:128, :])
            allred = statp.tile([128, 32], F32, tag="allred")
            nc.gpsimd.partition_all_reduce(
                allred[:], statb2[:], channels=128,
                reduce_op=bass_isa.ReduceOp.add)
            m = statp.tile([128, 8], F32, tag="m")
            e2 = statp.tile([128, 8], F32, tag="e2")
            msq = statp.tile([128, 8], F32, tag="msq")
            var = statp.tile([128, 8], F32, tag="var")
            sd = statp.tile([128, 8], F32, tag="sd")
            inv = statp.tile([128, 8], F32, tag="inv")
            st["s_t"] = s_t = statp.tile([128, 8], F32, tag="s_t", name=f"s_t{b}")
            tmp = statp.tile([128, 8], F32, tag="tmp")
            t_t = statp.tile([128, 8], F32, tag="t_t")
            for par in range(2):
                sl = slice(64 * par, 64 * (par + 1))
                co = 16 * par
                nc.scalar.mul(m[sl, :], allred[sl, co:co + 8], 1.0 / NGRP)
                nc.scalar.mul(e2[sl, :], allred[sl, co + 8:co + 16],
                              1.0 / NGRP)
                nc.scalar.activation(msq[sl, :], m[sl, :], ACTF.Square)
                nc.vector.tensor_tensor(var[sl, :], e2[sl, :], msq[sl, :],
                                        op=ALU.subtract)
                nc.scalar.activation(sd[sl, :], var[sl, :], ACTF.Sqrt,
                                     bias=epsb[sl, 0:1])
                nc.vector.reciprocal(inv[sl, :], sd[sl, :])
                nc.vector.tensor_tensor(s_t[sl, :], inv[sl, :], gw[sl, :],
                                        op=ALU.mult)
                nc.vector.tensor_tensor(tmp[sl, :], m[sl, :], s_t[sl, :],
                                        op=ALU.mult)
                nc.vector.tensor_tensor(t_t[sl, :], gb[sl, :], tmp[sl, :],
                                        op=ALU.subtract)
            # c = sum_h t_h * wo_hs[h]: move t to partitions, then matmul
            todd = statp.tile([1, 8], F32, tag="todd")
            nc.scalar.copy(todd[:], t_t[64:65, :])
            tvec = statp.tile([64, 1], BF16, tag="tvec")
            nc.gpsimd.memset(tvec[:], 0.0)
            tvp = ptv.tile([8, 2], F32, tag="tv", name=f"tv{b}")
            for hf, src in ((0, t_t[0:1, :]), (1, todd[:])):
                nc.tensor.matmul(tvp[:, hf:hf + 1], src, ones11[:],
                                 start=True, stop=True)
                nc.vector.tensor_copy(tvec[32 * hf:32 * hf + 8, :],
                                      tvp[:, hf:hf + 1])
            if DEBUG and b == 0:
                nc.sync.dma_start(dbgf_d[0:128, 0:16], statb[:])
                nc.sync.dma_start(dbgf_d[128:256, 0:32], allred[:])
                nc.sync.dma_start(dbgf_d[256:384, 0:8], s_t[:])
                nc.sync.dma_start(dbgf_d[384:512, 0:8], t_t[:])
                nc.sync.dma_start(dbgb_d[896:1024, 0:TPI], st["of"][0][:])
                nc.sync.dma_start(dbgb_d[1345:1409, 0:1], tvec[:])
            st["c_sb"] = c_sb = statp.tile([1, HID], BF16, tag="c_sb", name=f"c_sb{b}")
            for nn in range(2):
                cp = ptv.tile([1, 512], F32, tag="c", name=f"c{b}_{nn}")
                nc.tensor.matmul(cp[:], tvec[:],
                                 wo_hs[:, 512 * nn:512 * (nn + 1)],
                                 start=True, stop=True)
                nc.vector.tensor_copy(c_sb[:, 512 * nn:512 * (nn + 1)], cp[:])
            if DEBUG and b == 0:
                nc.sync.dma_start(dbgb_d[1344:1345, :], c_sb[:])
                nc.sync.dma_start(dbgb_d[1024:1152, :], wo_sb[0][:])
                nc.sync.dma_start(dbgb_d[1152:1280, :],
                                  wqk_sb[0][:, 0:HID])
                nc.sync.dma_start(dbgb_d[1280:1344, :], wo_hs[:])

        def emit_c_back(b, st):
            s_t, c_sb = st["s_t"], st["c_sb"]
            of2 = []
            for p in range(8):
                o2 = st["of"][p]
                nc.vector.tensor_scalar_mul(o2[:], o2[:], s_t[:, p:p + 1])
                of2.append(o2)
            for jj in range(JPI):
                ts = slice(TT * jj, TT * (jj + 1))
                y_sb = youtp.tile([TT, HID], F32, tag="y_sb",
                                  name=f"ysb{b}_{jj}")
                for nn in range(2):
                    cs = slice(512 * nn, 512 * (nn + 1))
                    yp = pyp.tile([TT, 512], F32, tag="yp")
                    for k in range(8):
                        nc.tensor.matmul(yp[:], of2[k][:, ts],
                                         wo_sb[k][:, cs],
                                         start=(k == 0), stop=False)
                    nc.tensor.matmul(yp[:], ones112[:], c_sb[:, cs],
                                     start=False, stop=True)
                    if nn == 0:
                        nc.scalar.copy(y_sb[:, cs], yp[:])
                    else:
                        nc.vector.tensor_copy(y_sb[:, cs], yp[:])
                nc.sync.dma_start(
                    y_d[TPI * b + TT * jj:TPI * b + TT * (jj + 1), :],
                    y_sb[:])

        with tc.tile_pool(name="tb", bufs=2) as tbp, \
             tc.tile_pool(name="of", bufs=2) as ofp, \
             tc.tile_pool(name="sq", bufs=2) as sqp, \
             tc.tile_pool(name="stat", bufs=2) as statp, \
             tc.tile_pool(name="yout", bufs=2) as youtp, \
             tc.tile_pool(name="pot", bufs=2, space="PSUM") as pot, \
             tc.tile_pool(name="pyp", bufs=2, space="PSUM") as pyp, \
             tc.tile_pool(name="ptv", bufs=1, space="PSUM") as ptv:
            cstates = {}
            for b in range(B_CORE + 1):
                if b < B_CORE:
                    cstates[b] = {}
                    emit_c_front(b, cstates[b])
                if b >= 1:
                    emit_c_back(b - 1, cstates.pop(b - 1))

    nc.compile()
    return nc


def _host_tables():
    import ml_dtypes
    inv_freq = 1.0 / (10000.0 ** (np.arange(0, 16, dtype=np.float64) * 2 / 32))
    wpos = np.arange(W, dtype=np.float64)
    ang = wpos[:, None] * inv_freq[None, :]          # [28, 16]
    cosw = np.cos(ang).astype(np.float32)
    sinw = np.sin(ang).astype(np.float32)
    cblk = np.concatenate([cosw, cosw], axis=1)       # [28, 32]
    sblk = np.concatenate([-sinw, sinw], axis=1)      # [28, 32]
    crow = np.tile(cblk, (1, HEADS))                  # [28, 512]
    srow = np.tile(sblk, (1, HEADS))
    ctab = np.tile(crow, (H, 1)).reshape(TPI, 512)
    stab = np.tile(srow, (H, 1)).reshape(TPI, 512)
    cst = np.concatenate([ctab, stab], axis=1).astype(ml_dtypes.bfloat16)
    at = np.zeros((TPI, 32), dtype=ml_dtypes.bfloat16)
    t = np.arange(TPI)
    at[t, t % W] = 1.0
    idnb = np.eye(128, dtype=ml_dtypes.bfloat16)
    hm = np.zeros((HID, 64), dtype=ml_dtypes.bfloat16)
    for k in range(8):
        hm[128 * k:128 * k + 64, k] = 1
        hm[128 * k + 64:128 * (k + 1), 32 + k] = 1
    return cst, at, idnb, hm


def kernel(x, input_img, qkv_w, o_w, gn_w, gn_b):
    x = np.ascontiguousarray(np.asarray(x, dtype=np.float32))
    input_img = np.ascontiguousarray(np.asarray(input_img, dtype=np.float32))
    qkv_w = np.asarray(qkv_w, dtype=np.float32)
    o_w = np.ascontiguousarray(np.asarray(o_w, dtype=np.float32))
    gn_w = np.asarray(gn_w, dtype=np.float32)
    gn_b = np.asarray(gn_b, dtype=np.float32)

    key = (tuple(gn_w.tolist()), tuple(gn_b.tolist()))
    if key not in _CACHE:
        _CACHE[key] = _build_program(gn_w, gn_b)
    nc = _CACHE[key]

    import ml_dtypes
    cst, at, idnb, hm = _host_tables()
    wqk = np.ascontiguousarray(np.concatenate(
        [qkv_w[:, 0:HID], qkv_w[:, 2 * HID:3 * HID]],
        axis=1)).astype(ml_dtypes.bfloat16)
    wv = np.ascontiguousarray(
        qkv_w[:, HID:2 * HID]).astype(ml_dtypes.bfloat16)

    in_maps = []
    for c in range(N_CORES):
        in_maps.append({
            "x": x[B_CORE * c:B_CORE * (c + 1)].reshape(
                TOK, HID).astype(ml_dtypes.bfloat16),
            "img": input_img[B_CORE * c:B_CORE * (c + 1)].reshape(
                TOK, HID).astype(ml_dtypes.bfloat16),
            "wqk": wqk, "wv": wv,
            "wo": o_w.astype(ml_dtypes.bfloat16),
            "at": at, "idnb": idnb,
            "cst": cst, "hm": hm,
        })
    res = run_bass_kernel_spmd(nc, in_maps, list(range(N_CORES)))
    out = np.concatenate(
        [res.results[c]["y"].reshape(B_CORE, H, W, HID)
         for c in range(N_CORES)], axis=0)
    return out

